# revision 19
# baseline (speedup 1.0000x reference)
"""BiAttention kernel for Trainium2 (Bass/Tile), 8-core data-parallel over batch.

Reference computation (per batch example):
    input_dot[l]  = input @ w_input                    [L]
    memory_dot[m] = memory @ w_memory                  [M]
    cross[l,m]    = (input * dot_scale) @ memory^T     [L,M]
    att = input_dot + memory_dot + cross
    att = where(mask_l | mask_m, -1e20, att)
    w1 = softmax_m(att); o1 = w1 @ memory
    w2 = softmax_l(max_m(att)); o2 = w2 @ input        [1,D]
    out = concat([input, o1, input*o1, o2*o1], -1)     [L,4D]

Key simplifications / engine mapping:
  * input_dot[l] is constant along m, so it cancels in softmax_m.  P is
    computed as exp(s1c[l] * (32*cross + 32*mdot - 32*BIGM*mask_m)) where
    s1c[l] = (1-mask_l)/32.  Masked-l rows get exp(0)=1 for every m ->
    uniform weights, matching the reference's softmax over an all-NEG row.
  * max_m(att) = input_dot[l] + log(max_m P_raw) for unmasked rows, so
    weight_two's logits come from ev = max(P_raw) * exp(input_dot-30) * s1.
  * 32*dot_scale is folded into the fp8 memory transpose (memT_f8); the
    input transpose is a plain cast.  The 32x pre-scale keeps fp8 operands
    out of the subnormal range; the Exp scale (s1/32) undoes it.
  * both big matmuls (scores and o1) run in fp8e4 DoubleRow at 2x PE rate.
    P is renormalized to [0,128] by its row max before the fp8 cast of P^T
    (raw exp values overflow fp8e4's +-240); compensated in the o1 scale.
    The mdot/mask score row stays bf16 (fp8 cannot hold -448*32).
  * input_dot itself is 4 tiny DR matmuls against an 8x-scaled fp8 copy of
    w_input (freeing the DVE); memory_dot is DVE row-dots + tiny column
    transposes onto partition 0.
  * example b+1's whole prologue (memory landing, bf16/fp8 casts, memT
    build, mdot/mask row) is emitted piecewise inside example b's l-loop so
    the PE never sits idle between examples; per-example tiles that must
    coexist across the boundary are double-buffered (exres pool).
  * block4 (o2*o1) uses an SBUF bf16 stash of o1; the sweep is split
    between the DVE and GpSimd engines.

Sharding: batch 16 -> 2 examples per core across 8 cores; D-sized vectors
replicated. Each core runs an identical NEFF on its own slice.
"""

import sys

sys.path.insert(0, "/opt/trn_rl_repo")

import numpy as np

import concourse.bass as bass
import concourse.tile as tile
from concourse import bacc, mybir
from concourse.bass import ds, ts
from concourse.bass_utils import run_bass_kernel_spmd

F32 = mybir.dt.float32
F32R = mybir.dt.float32r
BF16 = mybir.dt.bfloat16
FP8 = mybir.dt.float8e4
U8 = mybir.dt.uint8
P = 128
BIGM = 448.0          # mask suppression logit
CS = 32.0             # fp8 pre-scale folded into memT_f8
PN = 128.0            # P renormalization ceiling for the fp8 P^T cast

Exp = mybir.ActivationFunctionType.Exp
Copy = mybir.ActivationFunctionType.Copy
X = mybir.AxisListType.X
MUL = mybir.AluOpType.mult
ADD = mybir.AluOpType.add
DR = mybir.MatmulPerfMode.DoubleRow


def _r(ap):
    return ap.bitcast(F32R)


def _f(ap):
    return ap.bitcast(F32)


def biattn_tile_kernel(tc, out_ap, inp_ap, mem_ap, msk_ap, w_in_ap, w_mem_ap,
                       dscale_ap, BPC, L, D, M, reps=1):
    nc = tc.nc
    KD = D // P            # d-chunks
    NLT = L // P           # l-tiles
    NMC = M // P           # m-chunks
    AC = 512               # att column chunk (PSUM bank limit for fp32)
    NAC = M // AC
    DC = 512
    ND2 = D // DC
    GK = 8                 # transposes batched per PSUM group copy

    ident_dram = nc.inline_tensor(np.eye(P, dtype=np.float32), name="identconst")
    ones_dram = nc.inline_tensor(np.ones((1, P), dtype=np.float32), name="onesconst")

    import contextlib
    ctx = contextlib.ExitStack()
    with ctx:
        # --- pools ---
        consts = ctx.enter_context(tc.tile_pool(name="consts", bufs=1))
        residents = ctx.enter_context(tc.tile_pool(name="residents", bufs=1))
        exres = ctx.enter_context(tc.tile_pool(name="exres", bufs=2))
        landpool = ctx.enter_context(tc.tile_pool(name="landpool", bufs=2))
        infpool = ctx.enter_context(tc.tile_pool(name="infpool", bufs=4))
        junkpool = ctx.enter_context(tc.tile_pool(name="junkpool", bufs=1))
        inpool = ctx.enter_context(tc.tile_pool(name="inpool", bufs=3))
        sitpool = ctx.enter_context(tc.tile_pool(name="sitpool", bufs=2))
        ppool = ctx.enter_context(tc.tile_pool(name="ppool", bufs=2))
        ptpool = ctx.enter_context(tc.tile_pool(name="ptpool", bufs=1))
        o1pool = ctx.enter_context(tc.tile_pool(name="o1pool", bufs=2))
        b3pool = ctx.enter_context(tc.tile_pool(name="b3pool", bufs=2))
        t4pool = ctx.enter_context(tc.tile_pool(name="t4pool", bufs=2))
        smalls = ctx.enter_context(tc.tile_pool(name="smalls", bufs=2))
        attps = ctx.enter_context(tc.tile_pool(name="attps", bufs=2, space="PSUM"))
        tpps = ctx.enter_context(tc.tile_pool(name="tpps", bufs=2, space="PSUM"))
        o1ps = ctx.enter_context(tc.tile_pool(name="o1ps", bufs=1, space="PSUM"))
        o2ps = ctx.enter_context(tc.tile_pool(name="o2ps", bufs=1, space="PSUM"))
        rowpool = ctx.enter_context(tc.tile_pool(name="rowpool", bufs=1))

        # --- constants ---
        ident_f32 = consts.tile([P, P], F32)
        nc.scalar.dma_start(out=ident_f32, in_=ident_dram.ap())
        ident_bf = consts.tile([P, P], BF16)
        nc.vector.tensor_copy(out=ident_bf, in_=ident_f32)
        ident_r = consts.tile([P, P], F32R)
        nc.scalar.dma_start(out=ident_r, in_=_r(ident_dram.ap()))
        ones_f32 = consts.tile([1, P], F32)
        nc.scalar.dma_start(out=ones_f32, in_=ones_dram.ap())
        ones_bf = consts.tile([1, P], BF16)     # K=1 stationary for extra row
        nc.vector.tensor_copy(out=ones_bf, in_=ones_f32)
        onecol_bf = consts.tile([P, 1], BF16)   # reduction helper
        nc.vector.memset(onecol_bf, 1.0)
        neg30 = consts.tile([P, 1], F32)
        nc.vector.memset(neg30, -30.0)

        # dot_scale*CS in d-major layout [P, KD] (per-partition scale for the
        # fp8 memT build)
        ds32_col = consts.tile([P, KD], F32)
        nc.scalar.dma_start(
            out=ds32_col,
            in_=bass.AP(tensor=dscale_ap.tensor, offset=dscale_ap.offset,
                        ap=[[1, P], [P, KD]]))
        nc.vector.tensor_scalar(out=ds32_col, in0=ds32_col, scalar1=CS,
                                scalar2=0.0, op0=MUL, op1=ADD)

        # 8*w_input in fp8 d-major pairs (for the PE input_dot); the 8x keeps
        # the fp8 values out of the subnormal range, undone in the ev Exp
        w8_f32 = consts.tile([P, KD], F32)
        nc.scalar.dma_start(
            out=w8_f32,
            in_=bass.AP(tensor=w_in_ap.tensor, offset=w_in_ap.offset,
                        ap=[[1, P], [P, KD]]))
        w8col = consts.tile([P, KD, 1], FP8)
        nc.vector.tensor_scalar(out=w8col[:, :, 0], in0=w8_f32, scalar1=8.0,
                                scalar2=0.0, op0=MUL, op1=ADD)

        # w_memory * CS broadcast on partitions (for the DVE mdot row-dots)
        w_land = landpool.tile([P, 2, D], F32, tag="land")  # const-setup only
        nc.scalar.dma_start(
            out=w_land[:, 0, :],
            in_=bass.AP(tensor=w_mem_ap.tensor, offset=w_mem_ap.offset,
                        ap=[[0, P]] + list(w_mem_ap.ap)))
        w_mem32_bf = consts.tile([P, D], BF16)
        nc.vector.tensor_scalar(out=w_mem32_bf, in0=w_land[:, 0, :], scalar1=CS,
                                scalar2=0.0, op0=MUL, op1=ADD)

        prev_stash = None   # (b, stash, o2b) pending block-4 sweep
        ex = {}             # bb -> per-example prologue tiles
        NPIECE = 9

        def emit_piece(bb, idx):
            """Emit prologue piece `idx` for example `bb` (idx 0..NPIECE-1).

            Per-pair structure: the landed f32 memory pair is consumed
            immediately (f32r transposes -> memT_f8 with the dot_scale fold;
            fp8 cast -> mem_f8; DVE row-dots -> mdot), so only 2-3 landing
            buffers are ever alive.
            """
            if bb >= BPC or idx >= NPIECE:
                return
            if bb not in ex:
                mem_f8 = exres.tile([P, NMC, D], FP8, tag="memf8")
                memT_f8 = exres.tile([P, KD, M], FP8, tag="memT8")
                mask_row = exres.tile([1, M], U8, tag="mrow")
                mask_cols = exres.tile([P, NLT], U8, tag="mcols")
                mdot_cols = exres.tile([P, NMC], F32, tag="mdcols")
                extra_row = exres.tile([1, M], BF16, tag="erow")
                ex[bb] = {
                    "lands": {},
                    "mem_f8": mem_f8,
                    "memT_f8": memT_f8,
                    "mask_row": mask_row,
                    "mask_cols": mask_cols,
                    "mdot_cols": mdot_cols,
                    "extra_row": extra_row,
                }
            st = ex[bb]

            def land_pair(mc2):
                if mc2 >= NMC // 2:
                    return
                land = landpool.tile([P, 2, D], F32R, tag="land")
                nc.sync.dma_start(
                    out=land,
                    in_=_r(bass.AP(tensor=mem_ap.tensor,
                                   offset=mem_ap.offset + (bb * M + mc2 * 2 * P) * D,
                                   ap=[[D, P], [P * D, 2], [1, D]])))
                st["lands"][mc2] = land

            def pair_work(i):
                land = st["lands"].pop(i)
                # fp8 memory resident (o1 rhs)
                nc.scalar.copy(out=st["mem_f8"][:, 2 * i:2 * i + 2, :],
                               in_=_f(land))
                # memT_f8 slice for this pair: 16 f32r transposes, cast with
                # the CS*dot_scale per-partition fold (split ACT/DVE)
                for kp in range(KD // 2):
                    tp = tpps.tile([P, 4 * P], F32R, tag="tp")
                    for t in range(4):
                        k, j = 2 * kp + t // 2, t % 2
                        nc.tensor.transpose(tp[:, ts(t, P)],
                                            land[:, j, ts(k, P)], ident_r)
                    for t2 in range(2):
                        k = 2 * kp + t2
                        dst = st["memT_f8"][:, k, ds(2 * i * P, 2 * P)]
                        if kp % 2 == 0:
                            nc.scalar.activation(
                                out=dst, in_=_f(tp)[:, ts(t2, 2 * P)],
                                func=Copy, scale=ds32_col[:, k:k + 1])
                        else:
                            nc.vector.tensor_scalar_mul(
                                out=dst, in0=_f(tp)[:, ts(t2, 2 * P)],
                                scalar1=ds32_col[:, k:k + 1])
                # memory_dot row-dots for the two m-chunks, then the
                # extra-row slice for this pair (mdot*CS - BIGM*CS*mask)
                for j in range(2):
                    junk2 = junkpool.tile([P, D], BF16, tag="junk")
                    nc.vector.tensor_tensor(out=junk2, in0=_f(land)[:, j, :],
                                            in1=w_mem32_bf, op=MUL)
                    nc.vector.reduce_sum(
                        out=st["mdot_cols"][:, 2 * i + j:2 * i + j + 1],
                        in_=junk2, axis=X)
                row_ps = attps.tile([1, 2 * P], F32, tag="att")
                for j in range(2):
                    nc.tensor.transpose(
                        row_ps[0:1, ds(j * P, P)],
                        st["mdot_cols"][:, 2 * i + j:2 * i + j + 1], ident_f32)
                mneg_c = smalls.tile([1, 2 * P], F32, tag="mnegc")
                nc.vector.tensor_scalar(
                    out=mneg_c, in0=st["mask_row"][0:1, ds(2 * i * P, 2 * P)],
                    scalar1=-BIGM * CS, scalar2=0.0, op0=MUL, op1=ADD)
                nc.vector.tensor_add(
                    out=st["extra_row"][0:1, ds(2 * i * P, 2 * P)],
                    in0=row_ps, in1=mneg_c)

            if idx == 0:
                nc.sync.dma_start(out=st["mask_row"], in_=msk_ap[bb:bb + 1, :])
                nc.sync.dma_start(
                    out=st["mask_cols"],
                    in_=bass.AP(tensor=msk_ap.tensor,
                                offset=msk_ap.offset + bb * L,
                                ap=[[1, P], [P, NLT]]))
                land_pair(0), land_pair(1)
            elif idx <= 8:
                pair_work(idx - 1)
                land_pair(idx + 1)

        for _rep in range(reps):
          ex.clear()
          for b in range(BPC):
            if b == 0:
                for idx in range(NPIECE):
                    emit_piece(0, idx)
            st = ex[b]
            mem_f8 = st["mem_f8"]
            memT_f8 = st["memT_f8"]
            mask_cols = st["mask_cols"]
            extra_row = st["extra_row"]

            preload = {}
            for plt in range(2):
                pin = infpool.tile([P, D], F32R, tag="inf32")
                nc.sync.dma_start(out=pin, in_=_r(inp_ap[b, ts(plt, P), :]))
                preload[plt] = pin

            # ---------- block-4 sweep of the previous example ----------
            if prev_stash is not None:
                pb, pstash, po2b = prev_stash
                for slt in range(NLT):
                    t4 = t4pool.tile([P, D], F32, tag="t4")
                    nc.gpsimd.tensor_tensor(out=t4, in0=pstash[:, slt, :],
                                            in1=po2b, op=MUL)
                    nc.gpsimd.dma_start(out=out_ap[pb, ts(slt, P), 3 * D:4 * D],
                                        in_=t4)

            # ---------- software-pipelined l-loop ----------
            stash = exres.tile([P, NLT, D], FP8, tag="stash")
            evall = residents.tile([P, NLT], BF16, tag="evall")
            o2_ps = o2ps.tile([1, D], F32, tag="o2")

            score_state = {}
            sit_state = {}
            ev_state = {}

            def emit_sit(lt):
                if lt >= NLT:
                    return
                if lt in preload:
                    in_f32 = preload.pop(lt)
                else:
                    in_f32 = infpool.tile([P, D], F32R, tag="inf32")
                    nc.sync.dma_start(out=in_f32, in_=_r(inp_ap[b, ts(lt, P), :]))
                in_bf = inpool.tile([P, D], BF16, tag="inbf")
                nc.vector.tensor_copy(out=in_bf, in_=_f(in_f32))
                # block 0 goes straight back out
                nc.gpsimd.dma_start(out=out_ap[b, ts(lt, P), 0:D], in_=_f(in_f32))
                # input transpose -> siT fp8 (plain cast; dot_scale lives in
                # memT).  f32r transposes read the raw f32 tile directly.
                siT = sitpool.tile([P, KD, P], FP8, tag="sit")
                for g in range(KD // 4):
                    tp = tpps.tile([P, 4 * P], F32R, tag="tp")
                    for i in range(4):
                        nc.tensor.transpose(tp[:, ts(i, P)],
                                            _r(in_f32)[:, ts(g * 4 + i, P)],
                                            ident_r)
                    nc.vector.tensor_copy(out=siT[:, g * 4:(g + 1) * 4, :],
                                          in_=_f(tp))
                sit_state[lt] = (in_f32, in_bf, siT)

            def emit_score(lt):
                if lt not in sit_state:
                    emit_sit(lt)
                in_f32, in_bf, siT = sit_state.pop(lt)
                mask_f = smalls.tile([P, 1], F32, tag="maskf")
                nc.vector.tensor_copy(out=mask_f, in_=mask_cols[:, lt:lt + 1])
                s1 = smalls.tile([P, 1], F32, tag="s1")
                nc.vector.tensor_scalar(out=s1, in0=mask_f, scalar1=-1.0,
                                        scalar2=1.0, op0=MUL, op1=ADD)
                s1c = smalls.tile([P, 1], F32, tag="s1c")
                nc.vector.tensor_scalar(out=s1c, in0=s1, scalar1=1.0 / CS,
                                        scalar2=0.0, op0=MUL, op1=ADD)

                # input_dot (8x-scaled) on the PE: 4 tiny DR matmuls
                idot_ps = attps.tile([P, 1], F32, tag="att")
                for g2 in range(KD // 2):
                    nc.tensor.matmul(idot_ps, siT[:, 2 * g2:2 * g2 + 2, :],
                                     w8col[:, 2 * g2:2 * g2 + 2, :],
                                     start=(g2 == 0), stop=(g2 == KD // 2 - 1),
                                     perf_mode=DR, skip_group_check=True)
                idot8 = smalls.tile([P, 1], F32, tag="idot")
                nc.vector.tensor_copy(out=idot8, in_=idot_ps)

                # scores -> P = exp(s1c * att32) chunk by chunk, from PSUM
                p_sb = ppool.tile([P, M], BF16, tag="psb")
                rsum = smalls.tile([P, NAC], F32, tag="rsum")
                cmax = smalls.tile([P, NAC], F32, tag="cmax")
                for c in range(NAC):
                    att_ps = attps.tile([P, AC], F32, tag="att")
                    for g2 in range(KD // 2):
                        nc.tensor.matmul(att_ps,
                                         siT[:, 2 * g2:2 * g2 + 2, :],
                                         memT_f8[:, 2 * g2:2 * g2 + 2,
                                                 ds(c * AC, AC)],
                                         start=(g2 == 0), stop=False,
                                         perf_mode=DR, skip_group_check=True)
                    nc.tensor.matmul(att_ps, ones_bf,
                                     extra_row[0:1, ds(c * AC, AC)],
                                     start=False, stop=True,
                                     skip_group_check=True)
                    nc.scalar.activation(out=p_sb[:, ds(c * AC, AC)], in_=att_ps,
                                         func=Exp, scale=s1c,
                                         accum_out=rsum[:, c:c + 1])
                    nc.vector.reduce_max(out=cmax[:, c:c + 1],
                                         in_=p_sb[:, ds(c * AC, AC)], axis=X)

                score_state[lt] = (in_f32, in_bf, p_sb, rsum, s1, idot8, cmax)

            def emit_score_tail(lt):
                (in_f32, in_bf, p_sb, rsum, s1, idot8, cmax) = score_state[lt]
                rowsum = smalls.tile([P, 1], F32, tag="rowsum")
                nc.vector.reduce_sum(out=rowsum, in_=rsum, axis=X)
                recip = smalls.tile([P, 1], F32, tag="recip")
                nc.vector.reciprocal(recip, rowsum)
                score_state[lt] = (in_f32, in_bf, p_sb, rsum, s1, idot8, recip,
                                   cmax)

            def emit_ev_o2(lt):
                in_bf, maxp, s1, idot8 = ev_state.pop(lt)
                # ev = max(P_raw) * exp(idot - 30) * s1  (logits for weight_two)
                h = smalls.tile([P, 1], F32, tag="h")
                nc.scalar.activation(out=h, in_=idot8, func=Exp, bias=neg30,
                                     scale=1.0 / 8.0)
                hs = smalls.tile([P, 1], F32, tag="hs")
                nc.vector.tensor_tensor(out=hs, in0=h, in1=s1, op=MUL)
                nc.vector.tensor_scalar(out=evall[:, lt:lt + 1], in0=maxp,
                                        scalar1=hs, scalar2=0.0,
                                        op0=MUL, op1=ADD)
                for dc in range(ND2):
                    nc.tensor.matmul(o2_ps[0:1, ds(dc * DC, DC)],
                                     evall[:, lt:lt + 1],
                                     in_bf[:, ds(dc * DC, DC)],
                                     start=(lt == 0), stop=(lt == NLT - 1))

            def emit_out(lt):
                lsl = ts(lt, P)
                (in_f32, in_bf, p_sb, rsum, s1, idot8, recip,
                 cmax) = score_state.pop(lt)
                maxp = smalls.tile([P, 1], F32, tag="maxp")
                nc.vector.reduce_max(out=maxp, in_=cmax, axis=X)
                ev_state[lt] = (in_bf, maxp, s1, idot8)
                # renormalize P to [0, PN] so the fp8 cast cannot overflow
                # (raw exp values reach ~e^40); compensated in the o1 scale
                mrec = smalls.tile([P, 1], F32, tag="mrec")
                nc.vector.reciprocal(mrec, maxp)
                nc.vector.tensor_scalar(out=p_sb, in0=p_sb, scalar1=mrec,
                                        scalar2=PN, op0=MUL, op1=MUL)

                # P^T via PE transposes, cast to fp8 on the PSUM->SBUF copy
                PT = ptpool.tile([P, NMC, P], FP8, tag="pt")
                for g in range(NMC // GK):
                    tp = tpps.tile([P, GK * P], BF16, tag="tp")
                    for i in range(GK):
                        nc.tensor.transpose(tp[:, ts(i, P)],
                                            p_sb[:, ts(g * GK + i, P)], ident_bf)
                    nc.scalar.copy(out=PT[:, g * GK:(g + 1) * GK, :], in_=tp)

                # output_one = (P @ memory) * recip * maxp / PN
                o1_psum = o1ps.tile([P, D], F32, tag="o1p")
                for mc2 in range(NMC // 2):
                    for dc in range(ND2):
                        nc.tensor.matmul(o1_psum[:, ds(dc * DC, DC)],
                                         PT[:, 2 * mc2:2 * mc2 + 2, :],
                                         mem_f8[:, 2 * mc2:2 * mc2 + 2,
                                                ds(dc * DC, DC)],
                                         start=(mc2 == 0),
                                         stop=(mc2 == NMC // 2 - 1),
                                         perf_mode=DR, skip_group_check=True)
                o1_sb = o1pool.tile([P, D], F32, tag="o1")
                combo = smalls.tile([P, 1], F32, tag="combo")
                nc.vector.tensor_tensor(out=combo, in0=recip, in1=maxp, op=MUL)
                nc.vector.tensor_scalar(out=combo, in0=combo, scalar1=1.0 / PN,
                                        scalar2=0.0, op0=MUL, op1=ADD)
                nc.scalar.activation(out=o1_sb, in_=o1_psum, func=Copy,
                                     scale=combo)
                nc.vector.tensor_copy(out=stash[:, lt, :], in_=o1_sb)

                nc.gpsimd.dma_start(out=out_ap[b, lsl, D:2 * D], in_=o1_sb)
                blk3 = b3pool.tile([P, D], F32, tag="b3")
                nc.vector.tensor_tensor(out=blk3, in0=_f(in_f32), in1=o1_sb, op=MUL)
                nc.gpsimd.dma_start(out=out_ap[b, lsl, 2 * D:3 * D], in_=blk3)

            def emit_preload(lt):
                if lt >= NLT or lt in preload:
                    return
                pin = infpool.tile([P, D], F32R, tag="inf32")
                nc.sync.dma_start(out=pin, in_=_r(inp_ap[b, ts(lt, P), :]))
                preload[lt] = pin

            # pipelined emission: score(t+1) is emitted before out(t) so the
            # PE can run the next tile's matmuls while softmax finishes; the
            # next example's prologue pieces ride along after each tile.
            emit_sit(0)
            emit_sit(1)
            emit_score(0)
            for lt in range(NLT):
                emit_sit(lt + 2)
                emit_preload(lt + 4)
                if lt + 1 < NLT:
                    emit_score(lt + 1)
                emit_score_tail(lt)
                emit_out(lt)
                emit_ev_o2(lt)
                if lt >= 1:
                    emit_piece(b + 1, lt - 1)

            # ---------- finalize output_two ----------
            colsum_ps = attps.tile([NLT, 1], F32, tag="att")
            nc.tensor.matmul(colsum_ps, evall, onecol_bf, start=True, stop=True)
            cs_bf = smalls.tile([NLT, 1], BF16, tag="csbf")
            nc.vector.tensor_copy(out=cs_bf, in_=colsum_ps)
            z2_ps = attps.tile([1, 1], F32, tag="att")
            nc.tensor.matmul(z2_ps, cs_bf, onecol_bf[0:NLT, 0:1],
                             start=True, stop=True)
            z2r = smalls.tile([1, 1], F32, tag="z2r")
            nc.vector.reciprocal(z2r, z2_ps)
            o2_bf = rowpool.tile([1, D], BF16, tag="o2bf")
            nc.scalar.activation(out=o2_bf, in_=o2_ps, func=Copy, scale=z2r)
            # broadcast o2 across partitions via a K=1 ones matmul (PE is
            # idle here); exact same bf16 values, no DRAM roundtrip
            o2b = residents.tile([P, D], BF16, tag="o2b")
            for dc in range(ND2):
                bc_ps = attps.tile([P, DC], F32, tag="att")
                nc.tensor.matmul(bc_ps, ones_bf,
                                 o2_bf[0:1, ds(dc * DC, DC)],
                                 start=True, stop=True)
                nc.scalar.copy(out=o2b[:, ds(dc * DC, DC)], in_=bc_ps)
            prev_stash = (b, stash, o2b)

        # ---------- tail block-4 sweep for the last example ----------
        pb, pstash, po2b = prev_stash
        for lt in range(NLT):
            t4 = t4pool.tile([P, D], F32, tag="t4")
            nc.vector.tensor_tensor(out=t4, in0=pstash[:, lt, :], in1=po2b,
                                    op=MUL)
            nc.gpsimd.dma_start(out=out_ap[pb, ts(lt, P), 3 * D:4 * D], in_=t4)


def build_module(BPC, L, D, M, enable_asserts=False, reps=1):
    nc = bacc.Bacc("TRN2", target_bir_lowering=False, debug=False,
                   enable_asserts=enable_asserts, num_devices=1)
    inp = nc.dram_tensor("input", (BPC, L, D), F32, kind="ExternalInput").ap()
    mem = nc.dram_tensor("memory", (BPC, M, D), F32, kind="ExternalInput").ap()
    msk = nc.dram_tensor("mask", (BPC, L), U8, kind="ExternalInput").ap()
    w_in = nc.dram_tensor("w_input", (D,), F32, kind="ExternalInput").ap()
    w_mem = nc.dram_tensor("w_memory", (D,), F32, kind="ExternalInput").ap()
    dsc = nc.dram_tensor("dot_scale", (D,), F32, kind="ExternalInput").ap()
    out = nc.dram_tensor("out", (BPC, L, 4 * D), F32, kind="ExternalOutput").ap()
    with tile.TileContext(nc) as tc:
        biattn_tile_kernel(tc, out, inp, mem, msk, w_in, w_mem, dsc,
                           BPC, L, D, M, reps=reps)
    nc.compile()
    return nc


_NC_CACHE = {}


def kernel(input, memory, mask, w_input, w_memory, dot_scale, trace=False):
    B, L, D = input.shape
    M = memory.shape[1]
    NCORES = 8
    BPC = B // NCORES
    key = (BPC, L, D, M)
    if key not in _NC_CACHE:
        _NC_CACHE[key] = build_module(*key)
    nc = _NC_CACHE[key]

    input = np.ascontiguousarray(np.asarray(input, dtype=np.float32))
    memory = np.ascontiguousarray(np.asarray(memory, dtype=np.float32))
    mask_u8 = np.ascontiguousarray(np.asarray(mask).astype(np.uint8))
    w_input = np.ascontiguousarray(np.asarray(w_input, dtype=np.float32))
    w_memory = np.ascontiguousarray(np.asarray(w_memory, dtype=np.float32))
    dot_scale = np.ascontiguousarray(np.asarray(dot_scale, dtype=np.float32))

    in_maps = []
    for c in range(NCORES):
        sl = slice(c * BPC, (c + 1) * BPC)
        in_maps.append({
            "input": input[sl], "memory": memory[sl], "mask": mask_u8[sl],
            "w_input": w_input, "w_memory": w_memory, "dot_scale": dot_scale,
        })
    res = run_bass_kernel_spmd(nc, in_maps, core_ids=list(range(NCORES)),
                               trace=trace)
    out = np.concatenate([res.results[c]["out"] for c in range(NCORES)], axis=0)
    if trace:
        kernel.last_exec_time_ns = res.exec_time_ns
        kernel.last_results = res
    return out


# revision 24
# speedup vs baseline: 1.5404x; 1.5404x over previous
"""BiAttention kernel for Trainium2 (Bass/Tile), 8-core data-parallel over batch.

Reference computation (per batch example):
    input_dot[l]  = input @ w_input                    [L]
    memory_dot[m] = memory @ w_memory                  [M]
    cross[l,m]    = (input * dot_scale) @ memory^T     [L,M]
    att = input_dot + memory_dot + cross
    att = where(mask_l | mask_m, -1e20, att)
    w1 = softmax_m(att); o1 = w1 @ memory
    w2 = softmax_l(max_m(att)); o2 = w2 @ input        [1,D]
    out = concat([input, o1, input*o1, o2*o1], -1)     [L,4D]

Key simplifications / engine mapping:
  * input_dot[l] is constant along m, so it cancels in softmax_m.  P is
    computed as exp(s1c[l] * (32*cross + 32*mdot - 32*BIGM*mask_m)) where
    s1c[l] = (1-mask_l)/32.  Masked-l rows get exp(0)=1 for every m ->
    uniform weights, matching the reference's softmax over an all-NEG row.
  * max_m(att) = input_dot[l] + log(max_m P_raw) for unmasked rows, so
    weight_two's logits come from ev = max(P_raw) * exp(input_dot-30) * s1.
  * 32*dot_scale is folded into the fp8 memory transpose (memT_f8); the
    input transpose is a plain cast.  The 32x pre-scale keeps fp8 operands
    out of the subnormal range; the Exp scale (s1/32) undoes it.
  * both big matmuls (scores and o1) run in fp8e4 DoubleRow at 2x PE rate.
    P is renormalized to [0,128] by its row max before the fp8 cast of P^T
    (raw exp values overflow fp8e4's +-240); compensated in the o1 scale.
    The mdot/mask score row stays bf16 (fp8 cannot hold -448*32).
  * input_dot itself is 4 tiny DR matmuls against an 8x-scaled fp8 copy of
    w_input (freeing the DVE); memory_dot is DVE row-dots + tiny column
    transposes onto partition 0.
  * example b+1's whole prologue (memory landing, bf16/fp8 casts, memT
    build, mdot/mask row) is emitted piecewise inside example b's l-loop so
    the PE never sits idle between examples; per-example tiles that must
    coexist across the boundary are double-buffered (exres pool).
  * block4 (o2*o1) uses an SBUF bf16 stash of o1; the sweep is split
    between the DVE and GpSimd engines.

Sharding: batch 16 -> 2 examples per core across 8 cores; D-sized vectors
replicated. Each core runs an identical NEFF on its own slice.
"""

import sys

sys.path.insert(0, "/opt/trn_rl_repo")

import numpy as np

import concourse.bass as bass
import concourse.tile as tile
from concourse import bacc, mybir
from concourse.bass import ds, ts
from concourse.bass_utils import run_bass_kernel_spmd

F32 = mybir.dt.float32
F32R = mybir.dt.float32r
BF16 = mybir.dt.bfloat16
FP8 = mybir.dt.float8e4
U8 = mybir.dt.uint8
P = 128
BIGM = 448.0          # mask suppression logit
CS = 32.0             # fp8 pre-scale folded into memT_f8
PN = 128.0            # P renormalization ceiling for the fp8 P^T cast

Exp = mybir.ActivationFunctionType.Exp
Copy = mybir.ActivationFunctionType.Copy
X = mybir.AxisListType.X
MUL = mybir.AluOpType.mult
ADD = mybir.AluOpType.add
DR = mybir.MatmulPerfMode.DoubleRow


def _r(ap):
    return ap.bitcast(F32R)


def _f(ap):
    return ap.bitcast(F32)


def biattn_tile_kernel(tc, out_ap, inp_ap, mem_ap, msk_ap, w_in_ap, w_mem_ap,
                       dscale_ap, BPC, L, D, M, reps=1):
    nc = tc.nc
    KD = D // P            # d-chunks
    NLT = L // P           # l-tiles
    NMC = M // P           # m-chunks
    AC = 512               # att column chunk (PSUM bank limit for fp32)
    NAC = M // AC
    DC = 512
    ND2 = D // DC
    GK = 8                 # transposes batched per PSUM group copy

    ident_dram = nc.inline_tensor(np.eye(P, dtype=np.float32), name="identconst")
    ones_dram = nc.inline_tensor(np.ones((1, P), dtype=np.float32), name="onesconst")

    import contextlib
    ctx = contextlib.ExitStack()
    with ctx:
        # --- pools ---
        consts = ctx.enter_context(tc.tile_pool(name="consts", bufs=1))
        residents = ctx.enter_context(tc.tile_pool(name="residents", bufs=1))
        exres = ctx.enter_context(tc.tile_pool(name="exres", bufs=2))
        landpool = ctx.enter_context(tc.tile_pool(name="landpool", bufs=2))
        infpool = ctx.enter_context(tc.tile_pool(name="infpool", bufs=4))
        junkpool = ctx.enter_context(tc.tile_pool(name="junkpool", bufs=1))
        inpool = ctx.enter_context(tc.tile_pool(name="inpool", bufs=3))
        sitpool = ctx.enter_context(tc.tile_pool(name="sitpool", bufs=2))
        ppool = ctx.enter_context(tc.tile_pool(name="ppool", bufs=2))
        ptpool = ctx.enter_context(tc.tile_pool(name="ptpool", bufs=1))
        o1pool = ctx.enter_context(tc.tile_pool(name="o1pool", bufs=2))
        b3pool = ctx.enter_context(tc.tile_pool(name="b3pool", bufs=2))
        t4pool = ctx.enter_context(tc.tile_pool(name="t4pool", bufs=2))
        smalls = ctx.enter_context(tc.tile_pool(name="smalls", bufs=2))
        attps = ctx.enter_context(tc.tile_pool(name="attps", bufs=2, space="PSUM"))
        tpps = ctx.enter_context(tc.tile_pool(name="tpps", bufs=2, space="PSUM"))
        o1ps = ctx.enter_context(tc.tile_pool(name="o1ps", bufs=1, space="PSUM"))
        o2ps = ctx.enter_context(tc.tile_pool(name="o2ps", bufs=1, space="PSUM"))
        rowpool = ctx.enter_context(tc.tile_pool(name="rowpool", bufs=1))

        # --- constants ---
        ident_f32 = consts.tile([P, P], F32)
        nc.scalar.dma_start(out=ident_f32, in_=ident_dram.ap())
        ident_bf = consts.tile([P, P], BF16)
        nc.vector.tensor_copy(out=ident_bf, in_=ident_f32)
        ident_r = consts.tile([P, P], F32R)
        nc.scalar.dma_start(out=ident_r, in_=_r(ident_dram.ap()))
        ones_f32 = consts.tile([1, P], F32)
        nc.scalar.dma_start(out=ones_f32, in_=ones_dram.ap())
        ones_bf = consts.tile([1, P], BF16)     # K=1 stationary for extra row
        nc.vector.tensor_copy(out=ones_bf, in_=ones_f32)
        onecol_bf = consts.tile([P, 1], BF16)   # reduction helper
        nc.vector.memset(onecol_bf, 1.0)
        neg30 = consts.tile([P, 1], F32)
        nc.vector.memset(neg30, -30.0)

        # dot_scale*CS in d-major layout [P, KD] (per-partition scale for the
        # fp8 memT build)
        ds32_col = consts.tile([P, KD], F32)
        nc.scalar.dma_start(
            out=ds32_col,
            in_=bass.AP(tensor=dscale_ap.tensor, offset=dscale_ap.offset,
                        ap=[[1, P], [P, KD]]))
        nc.vector.tensor_scalar(out=ds32_col, in0=ds32_col, scalar1=CS,
                                scalar2=0.0, op0=MUL, op1=ADD)

        # 8*w_input in fp8 d-major pairs (for the PE input_dot); the 8x keeps
        # the fp8 values out of the subnormal range, undone in the ev Exp
        w8_f32 = consts.tile([P, KD], F32)
        nc.scalar.dma_start(
            out=w8_f32,
            in_=bass.AP(tensor=w_in_ap.tensor, offset=w_in_ap.offset,
                        ap=[[1, P], [P, KD]]))
        w8col = consts.tile([P, KD, 1], FP8)
        nc.vector.tensor_scalar(out=w8col[:, :, 0], in0=w8_f32, scalar1=8.0,
                                scalar2=0.0, op0=MUL, op1=ADD)

        # w_memory * CS broadcast on partitions (for the DVE mdot row-dots)
        w_land = landpool.tile([P, 2, D], F32, tag="land")  # const-setup only
        nc.scalar.dma_start(
            out=w_land[:, 0, :],
            in_=bass.AP(tensor=w_mem_ap.tensor, offset=w_mem_ap.offset,
                        ap=[[0, P]] + list(w_mem_ap.ap)))
        w_mem32_bf = consts.tile([P, D], BF16)
        nc.vector.tensor_scalar(out=w_mem32_bf, in0=w_land[:, 0, :], scalar1=CS,
                                scalar2=0.0, op0=MUL, op1=ADD)

        prev_stash = None   # (b, stash, o2b) pending block-4 sweep
        ex = {}             # bb -> per-example prologue tiles
        NPIECE = 9

        def emit_piece(bb, idx):
            """Emit prologue piece `idx` for example `bb` (idx 0..NPIECE-1).

            Per-pair structure: the landed f32 memory pair is consumed
            immediately (f32r transposes -> memT_f8 with the dot_scale fold;
            fp8 cast -> mem_f8; DVE row-dots -> mdot), so only 2-3 landing
            buffers are ever alive.
            """
            if bb >= BPC or idx >= NPIECE:
                return
            if bb not in ex:
                mem_f8 = exres.tile([P, NMC, D], FP8, tag="memf8")
                memT_f8 = exres.tile([P, KD, M], FP8, tag="memT8")
                mask_row = exres.tile([1, M], U8, tag="mrow")
                mask_cols = exres.tile([P, NLT], U8, tag="mcols")
                mdot_cols = exres.tile([P, NMC], F32, tag="mdcols")
                extra_row = exres.tile([1, M], BF16, tag="erow")
                ex[bb] = {
                    "lands": {},
                    "mem_f8": mem_f8,
                    "memT_f8": memT_f8,
                    "mask_row": mask_row,
                    "mask_cols": mask_cols,
                    "mdot_cols": mdot_cols,
                    "extra_row": extra_row,
                }
            st = ex[bb]

            def land_pair(mc2):
                if mc2 >= NMC // 2:
                    return
                land = landpool.tile([P, 2, D], F32R, tag="land")
                nc.sync.dma_start(
                    out=land,
                    in_=_r(bass.AP(tensor=mem_ap.tensor,
                                   offset=mem_ap.offset + (bb * M + mc2 * 2 * P) * D,
                                   ap=[[D, P], [P * D, 2], [1, D]])))
                st["lands"][mc2] = land

            def pair_work(i):
                land = st["lands"].pop(i)
                # fp8 memory resident (o1 rhs)
                nc.scalar.copy(out=st["mem_f8"][:, 2 * i:2 * i + 2, :],
                               in_=_f(land))
                # memT_f8 slice for this pair: 16 f32r transposes, cast with
                # the CS*dot_scale per-partition fold (split ACT/DVE)
                for kp in range(KD // 2):
                    tp = tpps.tile([P, 4 * P], F32R, tag="tp")
                    for t in range(4):
                        k, j = 2 * kp + t // 2, t % 2
                        nc.tensor.transpose(tp[:, ts(t, P)],
                                            land[:, j, ts(k, P)], ident_r)
                    for t2 in range(2):
                        k = 2 * kp + t2
                        dst = st["memT_f8"][:, k, ds(2 * i * P, 2 * P)]
                        if kp % 2 == 0:
                            nc.scalar.activation(
                                out=dst, in_=_f(tp)[:, ts(t2, 2 * P)],
                                func=Copy, scale=ds32_col[:, k:k + 1])
                        else:
                            nc.vector.tensor_scalar_mul(
                                out=dst, in0=_f(tp)[:, ts(t2, 2 * P)],
                                scalar1=ds32_col[:, k:k + 1])
                # memory_dot row-dots for the two m-chunks, then the
                # extra-row slice for this pair (mdot*CS - BIGM*CS*mask)
                for j in range(2):
                    junk2 = junkpool.tile([P, D], BF16, tag="junk")
                    nc.vector.tensor_tensor(out=junk2, in0=_f(land)[:, j, :],
                                            in1=w_mem32_bf, op=MUL)
                    nc.vector.reduce_sum(
                        out=st["mdot_cols"][:, 2 * i + j:2 * i + j + 1],
                        in_=junk2, axis=X)
                row_ps = attps.tile([1, 2 * P], F32, tag="att")
                for j in range(2):
                    nc.tensor.transpose(
                        row_ps[0:1, ds(j * P, P)],
                        st["mdot_cols"][:, 2 * i + j:2 * i + j + 1], ident_f32)
                mneg_c = rowpool.tile([1, 2 * P], F32, tag="mnegc")
                nc.vector.tensor_scalar(
                    out=mneg_c, in0=st["mask_row"][0:1, ds(2 * i * P, 2 * P)],
                    scalar1=-BIGM * CS, scalar2=0.0, op0=MUL, op1=ADD)
                nc.vector.tensor_add(
                    out=st["extra_row"][0:1, ds(2 * i * P, 2 * P)],
                    in0=row_ps, in1=mneg_c)

            if idx == 0:
                nc.sync.dma_start(out=st["mask_row"], in_=msk_ap[bb:bb + 1, :])
                nc.sync.dma_start(
                    out=st["mask_cols"],
                    in_=bass.AP(tensor=msk_ap.tensor,
                                offset=msk_ap.offset + bb * L,
                                ap=[[1, P], [P, NLT]]))
                land_pair(0), land_pair(1)
            elif idx <= 8:
                pair_work(idx - 1)
                land_pair(idx + 1)

        for _rep in range(reps):
          ex.clear()
          for b in range(BPC):
            if b == 0:
                for idx in range(NPIECE):
                    emit_piece(0, idx)
            st = ex[b]
            mem_f8 = st["mem_f8"]
            memT_f8 = st["memT_f8"]
            mask_cols = st["mask_cols"]
            extra_row = st["extra_row"]

            preload = {}
            for plt in range(2):
                pin = infpool.tile([P, D], F32R, tag="inf32")
                nc.sync.dma_start(out=pin, in_=_r(inp_ap[b, ts(plt, P), :]))
                preload[plt] = pin

            # ---------- block-4 sweep of the previous example ----------
            if prev_stash is not None:
                pb, pstash, po2b = prev_stash
                for slt in range(NLT):
                    t4 = t4pool.tile([P, D], F32, tag="t4")
                    nc.gpsimd.tensor_tensor(out=t4, in0=pstash[:, slt, :],
                                            in1=po2b, op=MUL)
                    nc.gpsimd.dma_start(out=out_ap[pb, ts(slt, P), 3 * D:4 * D],
                                        in_=t4)

            # ---------- software-pipelined l-loop ----------
            stash = exres.tile([P, NLT, D], FP8, tag="stash")
            evall = residents.tile([P, NLT], BF16, tag="evall")
            o2_ps = o2ps.tile([1, D], F32, tag="o2")

            score_state = {}
            sit_state = {}
            ev_state = {}

            def emit_sit(lt):
                if lt >= NLT:
                    return
                if lt in preload:
                    in_f32 = preload.pop(lt)
                else:
                    in_f32 = infpool.tile([P, D], F32R, tag="inf32")
                    nc.sync.dma_start(out=in_f32, in_=_r(inp_ap[b, ts(lt, P), :]))
                in_bf = inpool.tile([P, D], BF16, tag="inbf")
                nc.vector.tensor_copy(out=in_bf, in_=_f(in_f32))
                # block 0 goes straight back out
                nc.gpsimd.dma_start(out=out_ap[b, ts(lt, P), 0:D], in_=_f(in_f32))
                # input transpose -> siT fp8 (plain cast; dot_scale lives in
                # memT).  f32r transposes read the raw f32 tile directly.
                siT = sitpool.tile([P, KD, P], FP8, tag="sit")
                for g in range(KD // 4):
                    tp = tpps.tile([P, 4 * P], F32R, tag="tp")
                    for i in range(4):
                        nc.tensor.transpose(tp[:, ts(i, P)],
                                            _r(in_f32)[:, ts(g * 4 + i, P)],
                                            ident_r)
                    nc.vector.tensor_copy(out=siT[:, g * 4:(g + 1) * 4, :],
                                          in_=_f(tp))
                sit_state[lt] = (in_f32, in_bf, siT)

            def emit_score(lt):
                if lt not in sit_state:
                    emit_sit(lt)
                in_f32, in_bf, siT = sit_state.pop(lt)
                mask_f = smalls.tile([P, 1], F32, tag="maskf")
                nc.vector.tensor_copy(out=mask_f, in_=mask_cols[:, lt:lt + 1])
                s1 = smalls.tile([P, 1], F32, tag="s1")
                nc.vector.tensor_scalar(out=s1, in0=mask_f, scalar1=-1.0,
                                        scalar2=1.0, op0=MUL, op1=ADD)
                s1c = smalls.tile([P, 1], F32, tag="s1c")
                nc.vector.tensor_scalar(out=s1c, in0=s1, scalar1=1.0 / CS,
                                        scalar2=0.0, op0=MUL, op1=ADD)

                # input_dot (8x-scaled) on the PE: 4 tiny DR matmuls
                idot_ps = attps.tile([P, 1], F32, tag="att")
                for g2 in range(KD // 2):
                    nc.tensor.matmul(idot_ps, siT[:, 2 * g2:2 * g2 + 2, :],
                                     w8col[:, 2 * g2:2 * g2 + 2, :],
                                     start=(g2 == 0), stop=(g2 == KD // 2 - 1),
                                     perf_mode=DR, skip_group_check=True)
                idot8 = smalls.tile([P, 1], F32, tag="idot")
                nc.vector.tensor_copy(out=idot8, in_=idot_ps)

                # scores -> P = exp(s1c * att32) chunk by chunk, from PSUM
                p_sb = ppool.tile([P, M], BF16, tag="psb")
                rsum = smalls.tile([P, NAC], F32, tag="rsum")
                cmax = smalls.tile([P, NAC], F32, tag="cmax")
                for c in range(NAC):
                    att_ps = attps.tile([P, AC], F32, tag="att")
                    for g2 in range(KD // 2):
                        nc.tensor.matmul(att_ps,
                                         siT[:, 2 * g2:2 * g2 + 2, :],
                                         memT_f8[:, 2 * g2:2 * g2 + 2,
                                                 ds(c * AC, AC)],
                                         start=(g2 == 0), stop=False,
                                         perf_mode=DR, skip_group_check=True)
                    nc.tensor.matmul(att_ps, ones_bf,
                                     extra_row[0:1, ds(c * AC, AC)],
                                     start=False, stop=True,
                                     skip_group_check=True)
                    nc.scalar.activation(out=p_sb[:, ds(c * AC, AC)], in_=att_ps,
                                         func=Exp, scale=s1c,
                                         accum_out=rsum[:, c:c + 1])
                    nc.vector.reduce_max(out=cmax[:, c:c + 1],
                                         in_=p_sb[:, ds(c * AC, AC)], axis=X)

                score_state[lt] = (in_f32, in_bf, p_sb, rsum, s1, idot8, cmax)

            def emit_score_tail(lt):
                (in_f32, in_bf, p_sb, rsum, s1, idot8, cmax) = score_state[lt]
                rowsum = smalls.tile([P, 1], F32, tag="rowsum")
                nc.vector.reduce_sum(out=rowsum, in_=rsum, axis=X)
                recip = smalls.tile([P, 1], F32, tag="recip")
                nc.vector.reciprocal(recip, rowsum)
                score_state[lt] = (in_f32, in_bf, p_sb, rsum, s1, idot8, recip,
                                   cmax)

            def emit_ev_o2(lt):
                in_bf, maxp, s1, idot8 = ev_state.pop(lt)
                # ev = max(P_raw) * exp(idot - 30) * s1  (logits for weight_two)
                h = smalls.tile([P, 1], F32, tag="h")
                nc.scalar.activation(out=h, in_=idot8, func=Exp, bias=neg30,
                                     scale=1.0 / 8.0)
                hs = smalls.tile([P, 1], F32, tag="hs")
                nc.vector.tensor_tensor(out=hs, in0=h, in1=s1, op=MUL)
                nc.vector.tensor_scalar(out=evall[:, lt:lt + 1], in0=maxp,
                                        scalar1=hs, scalar2=0.0,
                                        op0=MUL, op1=ADD)
                for dc in range(ND2):
                    nc.tensor.matmul(o2_ps[0:1, ds(dc * DC, DC)],
                                     evall[:, lt:lt + 1],
                                     in_bf[:, ds(dc * DC, DC)],
                                     start=(lt == 0), stop=(lt == NLT - 1))

            def emit_out(lt):
                lsl = ts(lt, P)
                (in_f32, in_bf, p_sb, rsum, s1, idot8, recip,
                 cmax) = score_state.pop(lt)
                maxp = smalls.tile([P, 1], F32, tag="maxp")
                nc.vector.reduce_max(out=maxp, in_=cmax, axis=X)
                ev_state[lt] = (in_bf, maxp, s1, idot8)
                # renormalize P to [0, PN] so the fp8 cast cannot overflow
                # (raw exp values reach ~e^40); compensated in the o1 scale
                mrec = smalls.tile([P, 1], F32, tag="mrec")
                nc.vector.reciprocal(mrec, maxp)
                nc.vector.tensor_scalar(out=p_sb, in0=p_sb, scalar1=mrec,
                                        scalar2=PN, op0=MUL, op1=MUL)

                # P^T via PE transposes, cast to fp8 on the PSUM->SBUF copy
                PT = ptpool.tile([P, NMC, P], FP8, tag="pt")
                for g in range(NMC // GK):
                    tp = tpps.tile([P, GK * P], BF16, tag="tp")
                    for i in range(GK):
                        nc.tensor.transpose(tp[:, ts(i, P)],
                                            p_sb[:, ts(g * GK + i, P)], ident_bf)
                    nc.scalar.copy(out=PT[:, g * GK:(g + 1) * GK, :], in_=tp)

                # output_one = (P @ memory) * recip * maxp / PN
                o1_psum = o1ps.tile([P, D], F32, tag="o1p")
                for mc2 in range(NMC // 2):
                    for dc in range(ND2):
                        nc.tensor.matmul(o1_psum[:, ds(dc * DC, DC)],
                                         PT[:, 2 * mc2:2 * mc2 + 2, :],
                                         mem_f8[:, 2 * mc2:2 * mc2 + 2,
                                                ds(dc * DC, DC)],
                                         start=(mc2 == 0),
                                         stop=(mc2 == NMC // 2 - 1),
                                         perf_mode=DR, skip_group_check=True)
                o1_sb = o1pool.tile([P, D], F32, tag="o1")
                combo = smalls.tile([P, 1], F32, tag="combo")
                nc.vector.tensor_tensor(out=combo, in0=recip, in1=maxp, op=MUL)
                nc.vector.tensor_scalar(out=combo, in0=combo, scalar1=1.0 / PN,
                                        scalar2=0.0, op0=MUL, op1=ADD)
                nc.scalar.activation(out=o1_sb, in_=o1_psum, func=Copy,
                                     scale=combo)
                nc.vector.tensor_copy(out=stash[:, lt, :], in_=o1_sb)

                nc.gpsimd.dma_start(out=out_ap[b, lsl, D:2 * D], in_=o1_sb)
                blk3 = b3pool.tile([P, D], F32, tag="b3")
                nc.vector.tensor_tensor(out=blk3, in0=_f(in_f32), in1=o1_sb, op=MUL)
                nc.gpsimd.dma_start(out=out_ap[b, lsl, 2 * D:3 * D], in_=blk3)

            def emit_preload(lt):
                if lt >= NLT or lt in preload:
                    return
                pin = infpool.tile([P, D], F32R, tag="inf32")
                nc.sync.dma_start(out=pin, in_=_r(inp_ap[b, ts(lt, P), :]))
                preload[lt] = pin

            # pipelined emission: score(t+1) is emitted before out(t) so the
            # PE can run the next tile's matmuls while softmax finishes; the
            # next example's prologue pieces ride along after each tile.
            emit_sit(0)
            emit_sit(1)
            emit_score(0)
            for lt in range(NLT):
                emit_sit(lt + 2)
                emit_preload(lt + 4)
                if lt + 1 < NLT:
                    emit_score(lt + 1)
                emit_score_tail(lt)
                emit_out(lt)
                emit_ev_o2(lt)
                if lt >= 1:
                    emit_piece(b + 1, lt - 1)

            # ---------- finalize output_two ----------
            colsum_ps = attps.tile([NLT, 1], F32, tag="att")
            nc.tensor.matmul(colsum_ps, evall, onecol_bf, start=True, stop=True)
            cs_bf = smalls.tile([NLT, 1], BF16, tag="csbf")
            nc.vector.tensor_copy(out=cs_bf, in_=colsum_ps)
            z2_ps = attps.tile([1, 1], F32, tag="att")
            nc.tensor.matmul(z2_ps, cs_bf, onecol_bf[0:NLT, 0:1],
                             start=True, stop=True)
            z2r = smalls.tile([1, 1], F32, tag="z2r")
            nc.vector.reciprocal(z2r, z2_ps)
            o2_bf = rowpool.tile([1, D], BF16, tag="o2bf")
            nc.scalar.activation(out=o2_bf, in_=o2_ps, func=Copy, scale=z2r)
            # broadcast o2 across partitions via a K=1 ones matmul (PE is
            # idle here); exact same bf16 values, no DRAM roundtrip
            o2b = residents.tile([P, D], BF16, tag="o2b")
            for dc in range(ND2):
                bc_ps = attps.tile([P, DC], F32, tag="att")
                nc.tensor.matmul(bc_ps, ones_bf,
                                 o2_bf[0:1, ds(dc * DC, DC)],
                                 start=True, stop=True)
                nc.scalar.copy(out=o2b[:, ds(dc * DC, DC)], in_=bc_ps)
            prev_stash = (b, stash, o2b)

        # ---------- tail block-4 sweep for the last example ----------
        pb, pstash, po2b = prev_stash
        for lt in range(NLT):
            t4 = t4pool.tile([P, D], F32, tag="t4")
            nc.vector.tensor_tensor(out=t4, in0=pstash[:, lt, :], in1=po2b,
                                    op=MUL)
            nc.gpsimd.dma_start(out=out_ap[pb, ts(lt, P), 3 * D:4 * D], in_=t4)


def build_module(BPC, L, D, M, enable_asserts=False, reps=1):
    nc = bacc.Bacc("TRN2", target_bir_lowering=False, debug=False,
                   enable_asserts=enable_asserts, num_devices=1)
    inp = nc.dram_tensor("input", (BPC, L, D), F32, kind="ExternalInput").ap()
    mem = nc.dram_tensor("memory", (BPC, M, D), F32, kind="ExternalInput").ap()
    msk = nc.dram_tensor("mask", (BPC, L), U8, kind="ExternalInput").ap()
    w_in = nc.dram_tensor("w_input", (D,), F32, kind="ExternalInput").ap()
    w_mem = nc.dram_tensor("w_memory", (D,), F32, kind="ExternalInput").ap()
    dsc = nc.dram_tensor("dot_scale", (D,), F32, kind="ExternalInput").ap()
    out = nc.dram_tensor("out", (BPC, L, 4 * D), F32, kind="ExternalOutput").ap()
    with tile.TileContext(nc) as tc:
        biattn_tile_kernel(tc, out, inp, mem, msk, w_in, w_mem, dsc,
                           BPC, L, D, M, reps=reps)
    nc.compile()
    return nc


_NC_CACHE = {}


def kernel(input, memory, mask, w_input, w_memory, dot_scale, trace=False):
    B, L, D = input.shape
    M = memory.shape[1]
    NCORES = 8
    BPC = B // NCORES
    key = (BPC, L, D, M)
    if key not in _NC_CACHE:
        _NC_CACHE[key] = build_module(*key)
    nc = _NC_CACHE[key]

    input = np.ascontiguousarray(np.asarray(input, dtype=np.float32))
    memory = np.ascontiguousarray(np.asarray(memory, dtype=np.float32))
    mask_u8 = np.ascontiguousarray(np.asarray(mask).astype(np.uint8))
    w_input = np.ascontiguousarray(np.asarray(w_input, dtype=np.float32))
    w_memory = np.ascontiguousarray(np.asarray(w_memory, dtype=np.float32))
    dot_scale = np.ascontiguousarray(np.asarray(dot_scale, dtype=np.float32))

    in_maps = []
    for c in range(NCORES):
        sl = slice(c * BPC, (c + 1) * BPC)
        in_maps.append({
            "input": input[sl], "memory": memory[sl], "mask": mask_u8[sl],
            "w_input": w_input, "w_memory": w_memory, "dot_scale": dot_scale,
        })
    res = run_bass_kernel_spmd(nc, in_maps, core_ids=list(range(NCORES)),
                               trace=trace)
    out = np.concatenate([res.results[c]["out"] for c in range(NCORES)], axis=0)
    if trace:
        kernel.last_exec_time_ns = res.exec_time_ns
        kernel.last_results = res
    return out
